# revision 27
# baseline (speedup 1.0000x reference)
"""Causal multi-head attention (dense transformer block) on 8 Trainium2 cores.

Problem: x[4, 2048, 1024], 16 heads, head_dim 64, causal softmax attention
with QKV + output projections (torch Linear layout weights).

Sharding: 8 cores = 4 batches x 2 head-groups (8 heads each).  Each core
computes QKV projection for its 8 heads, attention, and its partial output
projection (row-parallel over w_out).  Host sums the two partials per batch
and adds b_out.

All device layouts are "transposed" so no on-device transposes are needed:
  - x is fed as xT [d, s]; Q^T/K^T are produced as [head_dim, s]
  - scores are computed as S^T [k, q]; the causal mask is folded into the
    score PSUM as an additive -160 block (identity-matmul accumulate), so
    exp(scale*(s-160)) ~ 0 on masked slots and no separate mask multiply
    is needed.
  - softmax denominators are accumulated on the vector engine (bf16 adds of
    the exp tiles) and collapsed with four 1-row ones-matmuls per head-pair
    boundary; this keeps the PE free of the per-k-tile ones-matmul streams.
  - O is accumulated directly as O^T [e_loc, q], which is exactly the lhsT
    the output projection needs.
Matmul inputs are bf16 (PSUM accumulation is fp32); everything else fp32.
"""

import sys

sys.path.insert(0, "/opt/trn_rl_repo")

import numpy as np
import ml_dtypes

import concourse.bass as bass
import concourse.mybir as mybir
import concourse.tile as tile
from concourse import bacc
from concourse import bass_utils
from concourse.masks import make_identity

F32 = mybir.dt.float32
BF16 = mybir.dt.bfloat16
EXP = mybir.ActivationFunctionType.Exp

B, S, D = 4, 2048, 1024
HTOT, HD = 16, 64
NCORES = 8
HLOC = HTOT // 2          # heads per core
ELOC = HLOC * HD          # 512 local embedding width
NHP = HLOC // 2           # 4 head pairs
QC = 512                  # q-chunk width
NQC = S // QC             # 4
NKT = S // 128            # 16 k tiles over sequence
NDT = D // 128            # 8 k tiles over model dim
SCALE = 1.0 / float(np.sqrt(HD))
MASKV = -160.0            # additive causal mask (exp(scale*-160) ~ 2e-9)

_CACHE = {}


def _build_nc():
    nc = bacc.Bacc("TRN2", target_bir_lowering=False, debug=False)

    xT = nc.dram_tensor("xT", [D, S], BF16, kind="ExternalInput")
    wqT = nc.dram_tensor("wqT", [D, ELOC], BF16, kind="ExternalInput")
    wkT = nc.dram_tensor("wkT", [D, ELOC], BF16, kind="ExternalInput")
    wvT = nc.dram_tensor("wvT", [D, ELOC], BF16, kind="ExternalInput")
    woT = nc.dram_tensor("woT", [ELOC, D], BF16, kind="ExternalInput")
    bqk = nc.dram_tensor("bqk", [128, 2, NHP], F32, kind="ExternalInput")
    bvb = nc.dram_tensor("bvb", [128, ELOC], F32, kind="ExternalInput")
    outp = nc.dram_tensor("outp", [S, D], F32, kind="ExternalOutput")

    with tile.TileContext(nc) as tc:
        with tc.tile_pool(name="const", bufs=1) as constp, \
             tc.tile_pool(name="wpool", bufs=1) as wp, \
             tc.tile_pool(name="qkv", bufs=1) as qkvp, \
             tc.tile_pool(name="xt", bufs=1) as xtp, \
             tc.tile_pool(name="pt", bufs=12) as ptp, \
             tc.tile_pool(name="acc", bufs=4) as accp, \
             tc.tile_pool(name="otn", bufs=8) as otnp, \
             tc.tile_pool(name="dr", bufs=6) as drp, \
             tc.tile_pool(name="osb", bufs=6) as osbp:

            # ---- constants (all gpsimd, no DMA deps) ----
            ident = constp.tile([128, 128], BF16, name="ident")
            make_identity(nc, ident[:])
            # additive causal mask in S^T [k, q] orientation: 0 where k <= q,
            # MASKV where k > q  (keep where -k + q >= 0)
            maskadd = constp.tile([128, 128], BF16, name="maskadd")
            nc.gpsimd.memset(maskadd[:], 0.0)
            nc.gpsimd.affine_select(
                out=maskadd[:], in_=maskadd[:],
                compare_op=mybir.AluOpType.is_ge,
                fill=MASKV, base=0,
                pattern=[[1, 128]], channel_multiplier=-1)
            # 32-wide ones block: col 0 sums the partition axis, cols 1-31
            # (value 1e-8) overwrite the other psd rows with tiny positives
            # so the boundary reciprocal never sees stale garbage (the psd
            # bank is shared with out-projection psums).
            ones_a = constp.tile([128, 32], BF16, name="ones_a")
            nc.gpsimd.memset(ones_a[:], 1e-8)
            nc.gpsimd.memset(ones_a[:, 0:1], 1.0)
            bc_x = constp.tile([98, 128], BF16, name="bc_x")
            nc.gpsimd.memset(bc_x[:], 0.0)
            nc.gpsimd.memset(bc_x[0:1, 0:64], 1.0)
            nc.gpsimd.memset(bc_x[32:33, 64:128], 1.0)
            bc_y = constp.tile([98, 128], BF16, name="bc_y")
            nc.gpsimd.memset(bc_y[:], 0.0)
            nc.gpsimd.memset(bc_y[64:65, 0:64], 1.0)
            nc.gpsimd.memset(bc_y[96:97, 64:128], 1.0)

            # ---- weights + xT.  Order matters: V weights and the first x
            # column-chunk land first so compute starts ~6us in; the rest of
            # x streams in column-chunks consumed st-by-st by make_v.  DMA
            # issue itself costs ~0.6us/descriptor per engine queue, so the
            # input loads round-robin across three issuing engines. ----
            _dma_engs = (nc.sync, nc.scalar, nc.gpsimd)
            _dma_n = [0]

            def dma_in(dst, src):
                _dma_engs[_dma_n[0] % 3].dma_start(dst, src)
                _dma_n[0] += 1

            bqk_sb = constp.tile([128, 2, NHP], F32, name="bqk_sb")
            dma_in(bqk_sb[:], bqk[:])
            bvb_sb = constp.tile([128, ELOC], F32, name="bvb_sb")
            dma_in(bvb_sb[:], bvb[:])

            wv_sb = []
            for kt in range(NDT):
                t = wp.tile([128, ELOC], BF16, name=f"wv{kt}")
                dma_in(t[:], wvT[128 * kt:128 * (kt + 1), :])
                wv_sb.append(t)
            xc = [[None] * NQC for _ in range(NDT)]
            for c in range(NQC):
                for kt in range(NDT):
                    t = xtp.tile([128, QC], BF16, name=f"x{kt}_{c}")
                    dma_in(
                        t[:], xT[128 * kt:128 * (kt + 1), QC * c:QC * (c + 1)])
                    xc[kt][c] = t
            wq_sb, wk_sb = [], []
            for kt in range(NDT):
                for lst, srct, nm in ((wq_sb, wqT, "wq"), (wk_sb, wkT, "wk")):
                    t = wp.tile([128, ELOC], BF16, name=f"{nm}{kt}")
                    dma_in(t[:], srct[128 * kt:128 * (kt + 1), :])
                    lst.append(t)
            wo_sb = []
            for hp in range(NHP):
                t = wp.tile([128, D], BF16, name=f"wo{hp}")
                dma_in(t[:], woT[128 * hp:128 * (hp + 1), :])
                wo_sb.append(t)

            # ---- QKV projection ----
            QT, KT = [], []
            for hp in range(NHP):
                QT.append(qkvp.tile([128, S], BF16, name=f"qt{hp}"))
                KT.append(qkvp.tile([128, S], BF16, name=f"kt{hp}"))
            V = [qkvp.tile([128, ELOC], BF16, name=f"v{st}") for st in range(NKT)]

            def make_v(psq, st):
                ps = psq.tile([128, ELOC], F32)
                c, o = st // 4, 128 * (st % 4)
                for kt in range(NDT):
                    nc.tensor.matmul(
                        ps[:],
                        lhsT=xc[kt][c][:, o:o + 128],
                        rhs=wv_sb[kt][:],
                        start=(kt == 0), stop=(kt == NDT - 1))
                nc.vector.tensor_add(V[st][:], ps[:], bvb_sb[:])

            def make_qtkt_chunk(ps, hp, qk, c):
                dst, wsb = (QT, wq_sb) if qk == 0 else (KT, wk_sb)
                for kt in range(NDT):
                    nc.tensor.matmul(
                        ps[:],
                        lhsT=wsb[kt][:, 128 * hp:128 * (hp + 1)],
                        rhs=xc[kt][c][:],
                        start=(kt == 0), stop=(kt == NDT - 1))
                nc.vector.tensor_scalar_add(
                    dst[hp][:, QC * c:QC * (c + 1)], ps[:],
                    bqk_sb[:, qk, hp:hp + 1])

            def make_qtkt(psq, hp):
                for qk in range(2):
                    for c in range(NQC):
                        ps = psq.tile([128, QC], F32)
                        make_qtkt_chunk(ps, hp, qk, c)

            with tc.tile_pool(name="psq", bufs=6, space="PSUM") as psq:
                # PE warmup: dummy matmuls with no DMA deps fill the initial
                # DMA wait and un-throttle the HAM clock gate early.
                for _ in range(64):
                    t = psq.tile([128, ELOC], F32, name="ps")
                    nc.tensor.matmul(t[:, 0:128], lhsT=ident[:],
                                     rhs=maskadd[:], start=True, stop=True)
                for st in range(NKT):
                    make_v(psq, st)
                make_qtkt(psq, 0)
                make_qtkt(psq, 1)

            # ---- attention + output projection ----
            # The whole attention runs as one flattened, software-pipelined
            # stream of (j, pair, i) steps with a 1-step skew: the score
            # matmuls + exp of step n+1 are issued before the AV matmuls of
            # step n, so the in-order PE queue never bubbles on the exp
            # feedback (bubbles re-throttle the HAM clock gate to 1.2 GHz).
            # The skew also spans stage boundaries, covering the softmax
            # normalization chain with the next stage's score matmuls.
            # Denominators: DVE accumulates the exp tiles per (pair, hp) in
            # bf16; at the stage boundary four 32-row ones-matmuls collapse
            # the partition axis into psd rows 0/32/64/96 (plus tiny positive
            # filler rows) for the reciprocal + broadcast.
            with tc.tile_pool(name="pss", bufs=2, space="PSUM") as pss, \
                 tc.tile_pool(name="psov", bufs=2, space="PSUM") as psov, \
                 tc.tile_pool(name="psd", bufs=1, space="PSUM") as psd, \
                 tc.tile_pool(name="psop", bufs=1, space="PSUM") as psop:

                state = {}   # (j, pair) -> dict(ps_ot, acc, otn)

                def emit_score(j, pair, i):
                    hps = (2 * pair, 2 * pair + 1)
                    diag = i >= 4 * j
                    w = 128 * (i - 4 * j) if diag else 0
                    pts = {}
                    for hp in hps:
                        ps_s = pss.tile([128, 2, QC], F32, tag="pss",
                                        name="ps_s")
                        pt = ptp.tile([128, 2, QC], BF16, tag="pt", name="pt")
                        for h2 in range(2):
                            nc.tensor.matmul(
                                ps_s[:, h2, w:QC],
                                lhsT=KT[hp][64 * h2:64 * (h2 + 1),
                                            128 * i:128 * (i + 1)],
                                rhs=QT[hp][64 * h2:64 * (h2 + 1),
                                           QC * j + w:QC * (j + 1)],
                                start=True, stop=not diag)
                            if diag:
                                # fold the causal mask into the score psum:
                                # += I.T @ maskadd = -160 above the diagonal
                                nc.tensor.matmul(
                                    ps_s[:, h2, w:w + 128],
                                    lhsT=ident[:], rhs=maskadd[:],
                                    start=False, stop=True,
                                    skip_group_check=True)
                        nc.scalar.activation(pt[:, :, w:QC], ps_s[:, :, w:QC],
                                             EXP, scale=SCALE)
                        pts[hp] = pt
                    return pts

                def emit_av(j, pair, i, pts):
                    hps = (2 * pair, 2 * pair + 1)
                    diag = i >= 4 * j
                    w = 128 * (i - 4 * j) if diag else 0
                    nkt = 4 * j + 4
                    last = (i == nkt - 1)
                    if i == 0:
                        state[(j, pair)] = {
                            "ps_ot": {hp: psov.tile([128, QC], F32,
                                                    tag="psov",
                                                    name=f"ps_ot{hp}")
                                      for hp in hps},
                            "acc": {hp: accp.tile([128, 2, QC], BF16,
                                                  tag="acc", name=f"acc{hp}")
                                    for hp in hps},
                        }
                    st = state[(j, pair)]
                    if last:
                        # the last k-tile's exp goes straight into the
                        # denominator psum at the boundary (PE accumulate),
                        # keeping the final DVE add off the critical path
                        st["pt_last"] = pts
                    for hp in hps:
                        pt = pts[hp]
                        if i == 0:
                            nc.vector.tensor_copy(st["acc"][hp][:], pt[:])
                        elif not last:
                            nc.vector.tensor_add(
                                st["acc"][hp][:, :, w:QC],
                                st["acc"][hp][:, :, w:QC], pt[:, :, w:QC])
                        for h2 in range(2):
                            nc.tensor.matmul(
                                st["ps_ot"][hp][64 * h2:64 * (h2 + 1), w:QC],
                                lhsT=V[i][:, 64 * (2 * hp + h2):
                                          64 * (2 * hp + h2 + 1)],
                                rhs=pt[:, h2, w:QC],
                                start=(i == 0), stop=last,
                                tile_position=(0, 64 * h2))

                def emit_boundary(j, pair):
                    # collapse acc along partitions into psd rows 0/32/64/96,
                    # reciprocal straight off PSUM, then per-hp broadcast
                    # matmuls reusing the psd bank.
                    hps = (2 * pair, 2 * pair + 1)
                    st = state.pop((j, pair))
                    ps_d = psd.tile([128, QC], F32, tag="psd", name="ps_d")
                    for ih, hp in enumerate(hps):
                        for h2 in range(2):
                            r = 64 * ih + 32 * h2
                            nc.tensor.matmul(
                                ps_d[r:r + 32, :], lhsT=ones_a[:],
                                rhs=st["acc"][hp][:, h2, :],
                                start=True, stop=False,
                                tile_position=(0, r))
                            nc.tensor.matmul(
                                ps_d[r:r + 32, 384:QC], lhsT=ones_a[:],
                                rhs=st["pt_last"][hp][:, h2, 384:QC],
                                start=False, stop=True,
                                tile_position=(0, r),
                                skip_group_check=True)
                    drf = drp.tile([98, QC], F32, name="drf")
                    nc.vector.reciprocal_approx_fast(drf[:], ps_d[0:98, :])
                    dr = drp.tile([98, QC], BF16, name="dr")
                    with nc.allow_low_precision(reason="denom bf16"):
                        nc.vector.tensor_copy(dr[:], drf[:])
                    for ih, hp in enumerate(hps):
                        bc = bc_x if ih == 0 else bc_y
                        nc.tensor.matmul(ps_d[:], lhsT=bc[:], rhs=dr[:],
                                         start=True, stop=True)
                        dbc = drp.tile([128, QC], BF16, name="dbc")
                        nc.vector.tensor_copy(dbc[:], ps_d[:])
                        otn = otnp.tile([128, QC], BF16, tag="otn", name="otn")
                        nc.vector.tensor_mul(otn[:], st["ps_ot"][hp][:],
                                             dbc[:])
                        otn_j[hp] = otn

                def out_proj(j):
                    # ping-pong the psum between the psop and (momentarily
                    # idle) psd banks so the PE never waits on a drain.
                    for m in range(4):
                        s0 = QC * j + 128 * m
                        for eo in range(2):
                            if (2 * m + eo) % 2 == 0:
                                ps_o = psop.tile([128, 512], F32, tag="ps",
                                                 name="ps_o")
                            else:
                                ps_o = psd.tile([128, 512], F32, tag="psd",
                                                name="ps_o")
                            for hp in range(NHP):
                                nc.tensor.matmul(
                                    ps_o[:],
                                    lhsT=otn_j[hp][:, 128 * m:128 * (m + 1)],
                                    rhs=wo_sb[hp][:, 512 * eo:512 * (eo + 1)],
                                    start=(hp == 0), stop=(hp == NHP - 1))
                            osb = osbp.tile([128, 512], F32)
                            if j == 0:
                                # tail: ACT is idle after the last exp
                                nc.scalar.copy(osb[:], ps_o[:])
                            else:
                                nc.vector.tensor_copy(osb[:], ps_o[:])
                            nc.sync.dma_start(
                                outp[s0:s0 + 128, 512 * eo:512 * (eo + 1)],
                                osb[:])

                seq = [(j, pair, i)
                       for j in (3, 2, 1, 0)
                       for pair in (0, 1)
                       for i in range(4 * j + 4)]
                otn_j = {}

                def retire(step):
                    # emit the AV matmuls of a completed score step, plus the
                    # stage-boundary work when it was the stage's last step
                    (pj, pp, pi), ppts = step
                    emit_av(pj, pp, pi, ppts)
                    if pi == 4 * pj + 3:
                        emit_boundary(pj, pp)
                        if pp == 1:
                            out_proj(pj)

                # Deferred QT/KT (hp 2,3) chunks, one per attention step of
                # the j=3 ramp: each chunk's psum drain is covered by a full
                # attention step and every step gets independent PE filler.
                # Ordered so pair 1's score matmuls find their chunks ready:
                # Q chunk 3 + K chunks 0-2 during (3,0), the rest in (3,1).
                qtkt_sched = {}
                _chunks = [(2, 0, 3), (3, 0, 3), (2, 1, 0), (3, 1, 0),
                           (2, 1, 1), (3, 1, 1), (2, 1, 2), (3, 1, 2),
                           (2, 1, 3), (3, 1, 3), (2, 0, 0), (3, 0, 0),
                           (2, 0, 1), (3, 0, 1), (2, 0, 2), (3, 0, 2)]
                _slots = [(3, 0, i) for i in range(8, 16)] + \
                         [(3, 1, i) for i in range(8)]
                for _slot, _ch in zip(_slots, _chunks):
                    qtkt_sched[_slot] = _ch

                from collections import deque
                pending = deque()
                SKEW = 1
                nchunk = [0]
                for stp in seq:
                    if stp in qtkt_sched:
                        hp_, qk_, c_ = qtkt_sched[stp]
                        if nchunk[0] % 2 == 0:
                            ps = psop.tile([128, QC], F32, tag="ps",
                                           name="ps_o")
                        else:
                            ps = psd.tile([128, QC], F32, tag="psd",
                                          name="ps_d")
                        nchunk[0] += 1
                        make_qtkt_chunk(ps, hp_, qk_, c_)
                    pts = emit_score(*stp)
                    pending.append((stp, pts))
                    if len(pending) > SKEW:
                        retire(pending.popleft())
                while pending:
                    retire(pending.popleft())
    nc.compile()
    return nc


def _get_nc():
    if "nc" not in _CACHE:
        _CACHE["nc"] = _build_nc()
    return _CACHE["nc"]


def _prep_core_inputs(x, w_qkv, b_qkv, w_out, b, hg):
    r0 = ELOC * hg
    wq = w_qkv[r0:r0 + ELOC, :]
    wk = w_qkv[D + r0:D + r0 + ELOC, :]
    wv = w_qkv[2 * D + r0:2 * D + r0 + ELOC, :]
    bq = b_qkv[r0:r0 + ELOC]
    bk = b_qkv[D + r0:D + r0 + ELOC]
    bv = b_qkv[2 * D + r0:2 * D + r0 + ELOC]

    bf = ml_dtypes.bfloat16
    bqk_arr = np.empty((128, 2, NHP), np.float32)
    bqk_arr[:, 0, :] = bq.reshape(NHP, 128).T
    bqk_arr[:, 1, :] = bk.reshape(NHP, 128).T
    return {
        "xT": np.ascontiguousarray(x[b].T).astype(bf),
        "wqT": np.ascontiguousarray(wq.T).astype(bf),
        "wkT": np.ascontiguousarray(wk.T).astype(bf),
        "wvT": np.ascontiguousarray(wv.T).astype(bf),
        "woT": np.ascontiguousarray(w_out[:, r0:r0 + ELOC].T).astype(bf),
        "bqk": bqk_arr,
        "bvb": np.tile(bv.astype(np.float32)[None, :], (128, 1)),
    }


def kernel(x, w_qkv, b_qkv, w_out, b_out, _trace=False, _trace_kwargs=None):
    x = np.asarray(x, np.float32)
    w_qkv = np.asarray(w_qkv, np.float32)
    b_qkv = np.asarray(b_qkv, np.float32)
    w_out = np.asarray(w_out, np.float32)
    b_out = np.asarray(b_out, np.float32)

    nc = _get_nc()
    in_maps = []
    for core in range(NCORES):
        b, hg = core // 2, core % 2
        in_maps.append(_prep_core_inputs(x, w_qkv, b_qkv, w_out, b, hg))

    kw = {}
    if _trace:
        kw.update(trace=True, **(_trace_kwargs or {}))
    import time
    res = None
    for attempt in range(4):
        try:
            res = bass_utils.run_bass_kernel_spmd(
                nc, in_maps, core_ids=list(range(NCORES)), **kw)
            break
        except Exception:
            if attempt == 3:
                raise
            # Transient axon/NRT device flake: reset the PJRT backend so the
            # retry starts from a clean client, like a fresh process would.
            try:
                import jax
                jax.clear_caches()
                import jax._src.xla_bridge as _xb
                _xb._clear_backends()
            except Exception:
                pass
            time.sleep(5.0 * (attempt + 1))

    out = np.empty((B, S, D), np.float32)
    for b in range(B):
        out[b] = res.results[2 * b]["outp"] + res.results[2 * b + 1]["outp"] \
            + b_out[None, :]
    if _trace:
        return out, res
    return out


# revision 29
# speedup vs baseline: 1.0073x; 1.0073x over previous
"""Causal multi-head attention (dense transformer block) on 8 Trainium2 cores.

Problem: x[4, 2048, 1024], 16 heads, head_dim 64, causal softmax attention
with QKV + output projections (torch Linear layout weights).

Sharding: 8 cores = 4 batches x 2 head-groups (8 heads each).  Each core
computes QKV projection for its 8 heads, attention, and its partial output
projection (row-parallel over w_out).  Host sums the two partials per batch
and adds b_out.

All device layouts are "transposed" so no on-device transposes are needed:
  - x is fed as xT [d, s]; Q^T/K^T are produced as [head_dim, s]
  - scores are computed as S^T [k, q]; the causal mask is folded into the
    score PSUM as an additive -160 block (identity-matmul accumulate), so
    exp(scale*(s-160)) ~ 0 on masked slots and no separate mask multiply
    is needed.
  - softmax denominators are accumulated on the vector engine (bf16 adds of
    the exp tiles) and collapsed with four 1-row ones-matmuls per head-pair
    boundary; this keeps the PE free of the per-k-tile ones-matmul streams.
  - O is accumulated directly as O^T [e_loc, q], which is exactly the lhsT
    the output projection needs.
Matmul inputs are bf16 (PSUM accumulation is fp32); everything else fp32.
"""

import sys

sys.path.insert(0, "/opt/trn_rl_repo")

import numpy as np
import ml_dtypes

import concourse.bass as bass
import concourse.mybir as mybir
import concourse.tile as tile
from concourse import bacc
from concourse import bass_utils
from concourse.masks import make_identity

F32 = mybir.dt.float32
BF16 = mybir.dt.bfloat16
EXP = mybir.ActivationFunctionType.Exp

B, S, D = 4, 2048, 1024
HTOT, HD = 16, 64
NCORES = 8
HLOC = HTOT // 2          # heads per core
ELOC = HLOC * HD          # 512 local embedding width
NHP = HLOC // 2           # 4 head pairs
QC = 512                  # q-chunk width
NQC = S // QC             # 4
NKT = S // 128            # 16 k tiles over sequence
NDT = D // 128            # 8 k tiles over model dim
SCALE = 1.0 / float(np.sqrt(HD))
MASKV = -160.0            # additive causal mask (exp(scale*-160) ~ 2e-9)

_CACHE = {}


def _build_nc():
    nc = bacc.Bacc("TRN2", target_bir_lowering=False, debug=False)

    xT = nc.dram_tensor("xT", [D, S], BF16, kind="ExternalInput")
    wqT = nc.dram_tensor("wqT", [D, ELOC], BF16, kind="ExternalInput")
    wkT = nc.dram_tensor("wkT", [D, ELOC], BF16, kind="ExternalInput")
    wvT = nc.dram_tensor("wvT", [D, ELOC], BF16, kind="ExternalInput")
    woT = nc.dram_tensor("woT", [ELOC, D], BF16, kind="ExternalInput")
    bqk = nc.dram_tensor("bqk", [128, 2, NHP], F32, kind="ExternalInput")
    bvb = nc.dram_tensor("bvb", [128, ELOC], F32, kind="ExternalInput")
    outp = nc.dram_tensor("outp", [S, D], F32, kind="ExternalOutput")

    with tile.TileContext(nc) as tc:
        with tc.tile_pool(name="const", bufs=1) as constp, \
             tc.tile_pool(name="wpool", bufs=1) as wp, \
             tc.tile_pool(name="qkv", bufs=1) as qkvp, \
             tc.tile_pool(name="xt", bufs=1) as xtp, \
             tc.tile_pool(name="pt", bufs=12) as ptp, \
             tc.tile_pool(name="acc", bufs=4) as accp, \
             tc.tile_pool(name="otn", bufs=8) as otnp, \
             tc.tile_pool(name="dr", bufs=6) as drp, \
             tc.tile_pool(name="osb", bufs=6) as osbp:

            # ---- constants (all gpsimd, no DMA deps) ----
            ident = constp.tile([128, 128], BF16, name="ident")
            make_identity(nc, ident[:])
            # additive causal mask in S^T [k, q] orientation: 0 where k <= q,
            # MASKV where k > q  (keep where -k + q >= 0)
            maskadd = constp.tile([128, 128], BF16, name="maskadd")
            nc.gpsimd.memset(maskadd[:], 0.0)
            nc.gpsimd.affine_select(
                out=maskadd[:], in_=maskadd[:],
                compare_op=mybir.AluOpType.is_ge,
                fill=MASKV, base=0,
                pattern=[[1, 128]], channel_multiplier=-1)
            # 32-wide ones block: col 0 sums the partition axis, cols 1-31
            # (value 1e-8) overwrite the other psd rows with tiny positives
            # so the boundary reciprocal never sees stale garbage (the psd
            # bank is shared with out-projection psums).
            ones_a = constp.tile([128, 32], BF16, name="ones_a")
            nc.gpsimd.memset(ones_a[:], 1e-8)
            nc.gpsimd.memset(ones_a[:, 0:1], 1.0)
            bc_x = constp.tile([98, 128], BF16, name="bc_x")
            nc.gpsimd.memset(bc_x[:], 0.0)
            nc.gpsimd.memset(bc_x[0:1, 0:64], 1.0)
            nc.gpsimd.memset(bc_x[32:33, 64:128], 1.0)
            bc_y = constp.tile([98, 128], BF16, name="bc_y")
            nc.gpsimd.memset(bc_y[:], 0.0)
            nc.gpsimd.memset(bc_y[64:65, 0:64], 1.0)
            nc.gpsimd.memset(bc_y[96:97, 64:128], 1.0)

            # ---- weights + xT.  Order matters: V weights and the first x
            # column-chunk land first so compute starts ~6us in; the rest of
            # x streams in column-chunks consumed st-by-st by make_v.  DMA
            # issue itself costs ~0.6us/descriptor per engine queue, so the
            # input loads round-robin across three issuing engines. ----
            _dma_engs = (nc.sync, nc.scalar, nc.gpsimd)
            _dma_n = [0]

            def dma_in(dst, src):
                _dma_engs[_dma_n[0] % 3].dma_start(dst, src)
                _dma_n[0] += 1

            bqk_sb = constp.tile([128, 2, NHP], F32, name="bqk_sb")
            dma_in(bqk_sb[:], bqk[:])
            bvb_sb = constp.tile([128, ELOC], F32, name="bvb_sb")
            dma_in(bvb_sb[:], bvb[:])

            wv_sb = []
            for kt in range(NDT):
                t = wp.tile([128, ELOC], BF16, name=f"wv{kt}")
                dma_in(t[:], wvT[128 * kt:128 * (kt + 1), :])
                wv_sb.append(t)
            xc = [[None] * NQC for _ in range(NDT)]
            for c in range(NQC):
                for kt in range(NDT):
                    t = xtp.tile([128, QC], BF16, name=f"x{kt}_{c}")
                    dma_in(
                        t[:], xT[128 * kt:128 * (kt + 1), QC * c:QC * (c + 1)])
                    xc[kt][c] = t
            wq_sb, wk_sb = [], []
            for kt in range(NDT):
                for lst, srct, nm in ((wq_sb, wqT, "wq"), (wk_sb, wkT, "wk")):
                    t = wp.tile([128, ELOC], BF16, name=f"{nm}{kt}")
                    dma_in(t[:], srct[128 * kt:128 * (kt + 1), :])
                    lst.append(t)
            wo_sb = []
            for hp in range(NHP):
                t = wp.tile([128, D], BF16, name=f"wo{hp}")
                dma_in(t[:], woT[128 * hp:128 * (hp + 1), :])
                wo_sb.append(t)

            # ---- QKV projection ----
            QT, KT = [], []
            for hp in range(NHP):
                QT.append(qkvp.tile([128, S], BF16, name=f"qt{hp}"))
                KT.append(qkvp.tile([128, S], BF16, name=f"kt{hp}"))
            V = [qkvp.tile([128, ELOC], BF16, name=f"v{st}") for st in range(NKT)]

            def make_v(psq, st):
                ps = psq.tile([128, ELOC], F32)
                c, o = st // 4, 128 * (st % 4)
                for kt in range(NDT):
                    nc.tensor.matmul(
                        ps[:],
                        lhsT=xc[kt][c][:, o:o + 128],
                        rhs=wv_sb[kt][:],
                        start=(kt == 0), stop=(kt == NDT - 1))
                nc.vector.tensor_add(V[st][:], ps[:], bvb_sb[:])

            def make_qtkt_chunk(ps, hp, qk, c):
                dst, wsb = (QT, wq_sb) if qk == 0 else (KT, wk_sb)
                for kt in range(NDT):
                    nc.tensor.matmul(
                        ps[:],
                        lhsT=wsb[kt][:, 128 * hp:128 * (hp + 1)],
                        rhs=xc[kt][c][:],
                        start=(kt == 0), stop=(kt == NDT - 1))
                nc.vector.tensor_scalar_add(
                    dst[hp][:, QC * c:QC * (c + 1)], ps[:],
                    bqk_sb[:, qk, hp:hp + 1])

            def make_qtkt(psq, hp):
                for qk in range(2):
                    for c in range(NQC):
                        ps = psq.tile([128, QC], F32)
                        make_qtkt_chunk(ps, hp, qk, c)

            with tc.tile_pool(name="psq", bufs=6, space="PSUM") as psq:
                # PE warmup: dummy matmuls with no DMA deps fill the initial
                # DMA wait and un-throttle the HAM clock gate early.
                for _ in range(64):
                    t = psq.tile([128, ELOC], F32, name="ps")
                    nc.tensor.matmul(t[:, 0:128], lhsT=ident[:],
                                     rhs=maskadd[:], start=True, stop=True)
                for st in range(NKT):
                    make_v(psq, st)
                make_qtkt(psq, 0)
                make_qtkt(psq, 1)

            # ---- attention + output projection ----
            # The whole attention runs as one flattened, software-pipelined
            # stream of (j, pair, i) steps with a 1-step skew: the score
            # matmuls + exp of step n+1 are issued before the AV matmuls of
            # step n, so the in-order PE queue never bubbles on the exp
            # feedback (bubbles re-throttle the HAM clock gate to 1.2 GHz).
            # The skew also spans stage boundaries, covering the softmax
            # normalization chain with the next stage's score matmuls.
            # Denominators: DVE accumulates the exp tiles per (pair, hp) in
            # bf16; at the stage boundary four 32-row ones-matmuls collapse
            # the partition axis into psd rows 0/32/64/96 (plus tiny positive
            # filler rows) for the reciprocal + broadcast.
            with tc.tile_pool(name="pss", bufs=2, space="PSUM") as pss, \
                 tc.tile_pool(name="psov", bufs=2, space="PSUM") as psov, \
                 tc.tile_pool(name="psd", bufs=1, space="PSUM") as psd, \
                 tc.tile_pool(name="psop", bufs=1, space="PSUM") as psop:

                state = {}   # (j, pair) -> dict(ps_ot, acc, otn)

                def emit_score(j, pair, i):
                    hps = (2 * pair, 2 * pair + 1)
                    diag = i >= 4 * j
                    w = 128 * (i - 4 * j) if diag else 0
                    pts = {}
                    for hp in hps:
                        ps_s = pss.tile([128, 2, QC], F32, tag="pss",
                                        name="ps_s")
                        pt = ptp.tile([128, 2, QC], BF16, tag="pt", name="pt")
                        for h2 in range(2):
                            nc.tensor.matmul(
                                ps_s[:, h2, w:QC],
                                lhsT=KT[hp][64 * h2:64 * (h2 + 1),
                                            128 * i:128 * (i + 1)],
                                rhs=QT[hp][64 * h2:64 * (h2 + 1),
                                           QC * j + w:QC * (j + 1)],
                                start=True, stop=not diag)
                            if diag:
                                # fold the causal mask into the score psum:
                                # += I.T @ maskadd = -160 above the diagonal
                                nc.tensor.matmul(
                                    ps_s[:, h2, w:w + 128],
                                    lhsT=ident[:], rhs=maskadd[:],
                                    start=False, stop=True,
                                    skip_group_check=True)
                        nc.scalar.activation(pt[:, :, w:QC], ps_s[:, :, w:QC],
                                             EXP, scale=SCALE)
                        pts[hp] = pt
                    return pts

                def emit_av(j, pair, i, pts):
                    hps = (2 * pair, 2 * pair + 1)
                    diag = i >= 4 * j
                    w = 128 * (i - 4 * j) if diag else 0
                    nkt = 4 * j + 4
                    last = (i == nkt - 1)
                    if i == 0:
                        state[(j, pair)] = {
                            "ps_ot": {hp: psov.tile([128, QC], F32,
                                                    tag="psov",
                                                    name=f"ps_ot{hp}")
                                      for hp in hps},
                            "acc": {hp: accp.tile([128, 2, QC], BF16,
                                                  tag="acc", name=f"acc{hp}")
                                    for hp in hps},
                        }
                    st = state[(j, pair)]
                    if last:
                        # the last k-tile's exp goes straight into the
                        # denominator psum at the boundary (PE accumulate),
                        # keeping the final DVE add off the critical path
                        st["pt_last"] = pts
                    for hp in hps:
                        pt = pts[hp]
                        if i == 0:
                            nc.vector.tensor_copy(st["acc"][hp][:], pt[:])
                        elif not last:
                            nc.vector.tensor_add(
                                st["acc"][hp][:, :, w:QC],
                                st["acc"][hp][:, :, w:QC], pt[:, :, w:QC])
                        for h2 in range(2):
                            nc.tensor.matmul(
                                st["ps_ot"][hp][64 * h2:64 * (h2 + 1), w:QC],
                                lhsT=V[i][:, 64 * (2 * hp + h2):
                                          64 * (2 * hp + h2 + 1)],
                                rhs=pt[:, h2, w:QC],
                                start=(i == 0), stop=last,
                                tile_position=(0, 64 * h2))

                def emit_boundary(j, pair):
                    # collapse acc along partitions into psd rows 0/32/64/96,
                    # reciprocal straight off PSUM, then per-hp broadcast
                    # matmuls reusing the psd bank.
                    hps = (2 * pair, 2 * pair + 1)
                    st = state.pop((j, pair))
                    ps_d = psd.tile([128, QC], F32, tag="psd", name="ps_d")
                    for ih, hp in enumerate(hps):
                        for h2 in range(2):
                            r = 64 * ih + 32 * h2
                            nc.tensor.matmul(
                                ps_d[r:r + 32, :], lhsT=ones_a[:],
                                rhs=st["acc"][hp][:, h2, :],
                                start=True, stop=False,
                                tile_position=(0, r))
                            nc.tensor.matmul(
                                ps_d[r:r + 32, 384:QC], lhsT=ones_a[:],
                                rhs=st["pt_last"][hp][:, h2, 384:QC],
                                start=False, stop=True,
                                tile_position=(0, r),
                                skip_group_check=True)
                    drf = drp.tile([98, QC], F32, name="drf")
                    nc.vector.reciprocal_approx_fast(drf[:], ps_d[0:98, :])
                    dr = drp.tile([98, QC], BF16, name="dr")
                    with nc.allow_low_precision(reason="denom bf16"):
                        nc.vector.tensor_copy(dr[:], drf[:])
                    for ih, hp in enumerate(hps):
                        bc = bc_x if ih == 0 else bc_y
                        nc.tensor.matmul(ps_d[:], lhsT=bc[:], rhs=dr[:],
                                         start=True, stop=True)
                        dbc = drp.tile([128, QC], BF16, name="dbc")
                        nc.vector.tensor_copy(dbc[:], ps_d[:])
                        otn = otnp.tile([128, QC], BF16, tag="otn", name="otn")
                        nc.vector.tensor_mul(otn[:], st["ps_ot"][hp][:],
                                             dbc[:])
                        otn_j[hp] = otn

                op_queue = []   # pending out-projection group closures

                def out_proj_group(j, m, eo, otns):
                    s0 = QC * j + 128 * m
                    if (2 * m + eo) % 2 == 0:
                        ps_o = psop.tile([128, 512], F32, tag="ps",
                                         name="ps_o")
                    else:
                        ps_o = psd.tile([128, 512], F32, tag="psd",
                                        name="ps_o")
                    for hp in range(NHP):
                        nc.tensor.matmul(
                            ps_o[:],
                            lhsT=otns[hp][:, 128 * m:128 * (m + 1)],
                            rhs=wo_sb[hp][:, 512 * eo:512 * (eo + 1)],
                            start=(hp == 0), stop=(hp == NHP - 1))
                    osb = osbp.tile([128, 512], F32)
                    if j == 0:
                        # tail: ACT is idle after the last exp
                        nc.scalar.copy(osb[:], ps_o[:])
                    else:
                        nc.vector.tensor_copy(osb[:], ps_o[:])
                    nc.sync.dma_start(
                        outp[s0:s0 + 128, 512 * eo:512 * (eo + 1)], osb[:])

                def out_proj(j):
                    # queue this q-chunk's projection groups; the main loop
                    # disperses one per attention step of the next stage so
                    # the PE gets independent filler between score and AV
                    # matmuls instead of a block with psum-drain stalls.
                    otns = dict(otn_j)
                    for m in range(4):
                        for eo in range(2):
                            op_queue.append((j, m, eo, otns))

                seq = [(j, pair, i)
                       for j in (3, 2, 1, 0)
                       for pair in (0, 1)
                       for i in range(4 * j + 4)]
                otn_j = {}

                def retire(step):
                    # emit the AV matmuls of a completed score step, plus the
                    # stage-boundary work when it was the stage's last step
                    (pj, pp, pi), ppts = step
                    emit_av(pj, pp, pi, ppts)
                    if pi == 4 * pj + 3:
                        emit_boundary(pj, pp)
                        if pp == 1:
                            out_proj(pj)

                # Deferred QT/KT (hp 2,3) chunks, one per attention step of
                # the j=3 ramp: each chunk's psum drain is covered by a full
                # attention step and every step gets independent PE filler.
                # Ordered so pair 1's score matmuls find their chunks ready:
                # Q chunk 3 + K chunks 0-2 during (3,0), the rest in (3,1).
                qtkt_sched = {}
                _chunks = [(2, 0, 3), (3, 0, 3), (2, 1, 0), (3, 1, 0),
                           (2, 1, 1), (3, 1, 1), (2, 1, 2), (3, 1, 2),
                           (2, 1, 3), (3, 1, 3), (2, 0, 0), (3, 0, 0),
                           (2, 0, 1), (3, 0, 1), (2, 0, 2), (3, 0, 2)]
                _slots = [(3, 0, i) for i in range(8, 16)] + \
                         [(3, 1, i) for i in range(8)]
                for _slot, _ch in zip(_slots, _chunks):
                    qtkt_sched[_slot] = _ch

                from collections import deque
                pending = deque()
                SKEW = 1
                nchunk = [0]
                for stp in seq:
                    if stp in qtkt_sched:
                        hp_, qk_, c_ = qtkt_sched[stp]
                        if nchunk[0] % 2 == 0:
                            ps = psop.tile([128, QC], F32, tag="ps",
                                           name="ps_o")
                        else:
                            ps = psd.tile([128, QC], F32, tag="psd",
                                          name="ps_d")
                        nchunk[0] += 1
                        make_qtkt_chunk(ps, hp_, qk_, c_)
                    pts = emit_score(*stp)
                    if op_queue:
                        out_proj_group(*op_queue.pop(0))
                    pending.append((stp, pts))
                    if len(pending) > SKEW:
                        retire(pending.popleft())
                while pending:
                    retire(pending.popleft())
                while op_queue:
                    out_proj_group(*op_queue.pop(0))
    nc.compile()
    return nc


def _get_nc():
    if "nc" not in _CACHE:
        _CACHE["nc"] = _build_nc()
    return _CACHE["nc"]


def _prep_core_inputs(x, w_qkv, b_qkv, w_out, b, hg):
    r0 = ELOC * hg
    wq = w_qkv[r0:r0 + ELOC, :]
    wk = w_qkv[D + r0:D + r0 + ELOC, :]
    wv = w_qkv[2 * D + r0:2 * D + r0 + ELOC, :]
    bq = b_qkv[r0:r0 + ELOC]
    bk = b_qkv[D + r0:D + r0 + ELOC]
    bv = b_qkv[2 * D + r0:2 * D + r0 + ELOC]

    bf = ml_dtypes.bfloat16
    bqk_arr = np.empty((128, 2, NHP), np.float32)
    bqk_arr[:, 0, :] = bq.reshape(NHP, 128).T
    bqk_arr[:, 1, :] = bk.reshape(NHP, 128).T
    return {
        "xT": np.ascontiguousarray(x[b].T).astype(bf),
        "wqT": np.ascontiguousarray(wq.T).astype(bf),
        "wkT": np.ascontiguousarray(wk.T).astype(bf),
        "wvT": np.ascontiguousarray(wv.T).astype(bf),
        "woT": np.ascontiguousarray(w_out[:, r0:r0 + ELOC].T).astype(bf),
        "bqk": bqk_arr,
        "bvb": np.tile(bv.astype(np.float32)[None, :], (128, 1)),
    }


def kernel(x, w_qkv, b_qkv, w_out, b_out, _trace=False, _trace_kwargs=None):
    x = np.asarray(x, np.float32)
    w_qkv = np.asarray(w_qkv, np.float32)
    b_qkv = np.asarray(b_qkv, np.float32)
    w_out = np.asarray(w_out, np.float32)
    b_out = np.asarray(b_out, np.float32)

    nc = _get_nc()
    in_maps = []
    for core in range(NCORES):
        b, hg = core // 2, core % 2
        in_maps.append(_prep_core_inputs(x, w_qkv, b_qkv, w_out, b, hg))

    kw = {}
    if _trace:
        kw.update(trace=True, **(_trace_kwargs or {}))
    import time
    res = None
    for attempt in range(4):
        try:
            res = bass_utils.run_bass_kernel_spmd(
                nc, in_maps, core_ids=list(range(NCORES)), **kw)
            break
        except Exception:
            if attempt == 3:
                raise
            # Transient axon/NRT device flake: reset the PJRT backend so the
            # retry starts from a clean client, like a fresh process would.
            try:
                import jax
                jax.clear_caches()
                import jax._src.xla_bridge as _xb
                _xb._clear_backends()
            except Exception:
                pass
            time.sleep(5.0 * (attempt + 1))

    out = np.empty((B, S, D), np.float32)
    for b in range(B):
        out[b] = res.results[2 * b]["outp"] + res.results[2 * b + 1]["outp"] \
            + b_out[None, :]
    if _trace:
        return out, res
    return out


# revision 30
# speedup vs baseline: 1.1918x; 1.1831x over previous
"""Causal multi-head attention (dense transformer block) on 8 Trainium2 cores.

Problem: x[4, 2048, 1024], 16 heads, head_dim 64, causal softmax attention
with QKV + output projections (torch Linear layout weights).

Sharding: 8 cores = 4 batches x 2 head-groups (8 heads each).  Each core
computes QKV projection for its 8 heads, attention, and its partial output
projection (row-parallel over w_out).  Host sums the two partials per batch
and adds b_out.

All device layouts are "transposed" so no on-device transposes are needed:
  - x is fed as xT [d, s]; Q^T/K^T are produced as [head_dim, s]
  - scores are computed as S^T [k, q]; the causal mask is folded into the
    score PSUM as an additive -160 block (identity-matmul accumulate), so
    exp(scale*(s-160)) ~ 0 on masked slots and no separate mask multiply
    is needed.
  - softmax denominators are accumulated on the vector engine (bf16 adds of
    the exp tiles) and collapsed with four 1-row ones-matmuls per head-pair
    boundary; this keeps the PE free of the per-k-tile ones-matmul streams.
  - O is accumulated directly as O^T [e_loc, q], which is exactly the lhsT
    the output projection needs.
Matmul inputs are bf16 (PSUM accumulation is fp32); everything else fp32.
"""

import sys

sys.path.insert(0, "/opt/trn_rl_repo")

import numpy as np
import ml_dtypes

import concourse.bass as bass
import concourse.mybir as mybir
import concourse.tile as tile
from concourse import bacc
from concourse import bass_utils
from concourse.masks import make_identity

F32 = mybir.dt.float32
BF16 = mybir.dt.bfloat16
EXP = mybir.ActivationFunctionType.Exp

B, S, D = 4, 2048, 1024
HTOT, HD = 16, 64
NCORES = 8
HLOC = HTOT // 2          # heads per core
ELOC = HLOC * HD          # 512 local embedding width
NHP = HLOC // 2           # 4 head pairs
QC = 512                  # q-chunk width
NQC = S // QC             # 4
NKT = S // 128            # 16 k tiles over sequence
NDT = D // 128            # 8 k tiles over model dim
SCALE = 1.0 / float(np.sqrt(HD))
MASKV = -160.0            # additive causal mask (exp(scale*-160) ~ 2e-9)

_CACHE = {}


def _build_nc():
    nc = bacc.Bacc("TRN2", target_bir_lowering=False, debug=False)

    xT = nc.dram_tensor("xT", [D, S], BF16, kind="ExternalInput")
    wqT = nc.dram_tensor("wqT", [D, ELOC], BF16, kind="ExternalInput")
    wkT = nc.dram_tensor("wkT", [D, ELOC], BF16, kind="ExternalInput")
    wvT = nc.dram_tensor("wvT", [D, ELOC], BF16, kind="ExternalInput")
    woT = nc.dram_tensor("woT", [ELOC, D], BF16, kind="ExternalInput")
    bqk = nc.dram_tensor("bqk", [128, 2, NHP], F32, kind="ExternalInput")
    bvb = nc.dram_tensor("bvb", [128, ELOC], F32, kind="ExternalInput")
    outp = nc.dram_tensor("outp", [S, D], F32, kind="ExternalOutput")

    with tile.TileContext(nc) as tc:
        with tc.tile_pool(name="const", bufs=1) as constp, \
             tc.tile_pool(name="wpool", bufs=1) as wp, \
             tc.tile_pool(name="qkv", bufs=1) as qkvp, \
             tc.tile_pool(name="xt", bufs=1) as xtp, \
             tc.tile_pool(name="pt", bufs=12) as ptp, \
             tc.tile_pool(name="acc", bufs=4) as accp, \
             tc.tile_pool(name="otn", bufs=8) as otnp, \
             tc.tile_pool(name="dr", bufs=6) as drp, \
             tc.tile_pool(name="osb", bufs=6) as osbp:

            # ---- constants (all gpsimd, no DMA deps) ----
            ident = constp.tile([128, 128], BF16, name="ident")
            make_identity(nc, ident[:])
            # additive causal mask in S^T [k, q] orientation: 0 where k <= q,
            # MASKV where k > q  (keep where -k + q >= 0)
            maskadd = constp.tile([128, 128], BF16, name="maskadd")
            nc.gpsimd.memset(maskadd[:], 0.0)
            nc.gpsimd.affine_select(
                out=maskadd[:], in_=maskadd[:],
                compare_op=mybir.AluOpType.is_ge,
                fill=MASKV, base=0,
                pattern=[[1, 128]], channel_multiplier=-1)
            # 32-wide ones block: col 0 sums the partition axis, cols 1-31
            # (value 1e-8) overwrite the other psd rows with tiny positives
            # so the boundary reciprocal never sees stale garbage (the psd
            # bank is shared with out-projection psums).
            ones_a = constp.tile([128, 32], BF16, name="ones_a")
            nc.gpsimd.memset(ones_a[:], 1e-8)
            nc.gpsimd.memset(ones_a[:, 0:1], 1.0)
            bc_x = constp.tile([98, 128], BF16, name="bc_x")
            nc.gpsimd.memset(bc_x[:], 0.0)
            nc.gpsimd.memset(bc_x[0:1, 0:64], 1.0)
            nc.gpsimd.memset(bc_x[32:33, 64:128], 1.0)
            bc_y = constp.tile([98, 128], BF16, name="bc_y")
            nc.gpsimd.memset(bc_y[:], 0.0)
            nc.gpsimd.memset(bc_y[64:65, 0:64], 1.0)
            nc.gpsimd.memset(bc_y[96:97, 64:128], 1.0)

            # ---- weights + xT.  Order matters: V weights and the first x
            # column-chunk land first so compute starts ~6us in; the rest of
            # x streams in column-chunks consumed st-by-st by make_v.  DMA
            # issue itself costs ~0.6us/descriptor per engine queue, so the
            # input loads round-robin across three issuing engines. ----
            _dma_engs = (nc.sync, nc.scalar, nc.gpsimd)
            _dma_n = [0]

            def dma_in(dst, src):
                _dma_engs[_dma_n[0] % 3].dma_start(dst, src)
                _dma_n[0] += 1

            bqk_sb = constp.tile([128, 2, NHP], F32, name="bqk_sb")
            dma_in(bqk_sb[:], bqk[:])
            bvb_sb = constp.tile([128, ELOC], F32, name="bvb_sb")
            dma_in(bvb_sb[:], bvb[:])

            wv_sb = []
            for kt in range(NDT):
                t = wp.tile([128, ELOC], BF16, name=f"wv{kt}")
                dma_in(t[:], wvT[128 * kt:128 * (kt + 1), :])
                wv_sb.append(t)
            xc = [[None] * NQC for _ in range(NDT)]
            for c in range(NQC):
                for kt in range(NDT):
                    t = xtp.tile([128, QC], BF16, name=f"x{kt}_{c}")
                    dma_in(
                        t[:], xT[128 * kt:128 * (kt + 1), QC * c:QC * (c + 1)])
                    xc[kt][c] = t
            wq_sb, wk_sb = [], []
            for kt in range(NDT):
                for lst, srct, nm in ((wq_sb, wqT, "wq"), (wk_sb, wkT, "wk")):
                    t = wp.tile([128, ELOC], BF16, name=f"{nm}{kt}")
                    dma_in(t[:], srct[128 * kt:128 * (kt + 1), :])
                    lst.append(t)
            wo_sb = []
            for hp in range(NHP):
                t = wp.tile([128, D], BF16, name=f"wo{hp}")
                dma_in(t[:], woT[128 * hp:128 * (hp + 1), :])
                wo_sb.append(t)

            # ---- QKV projection ----
            QT, KT = [], []
            for hp in range(NHP):
                QT.append(qkvp.tile([128, S], BF16, name=f"qt{hp}"))
                KT.append(qkvp.tile([128, S], BF16, name=f"kt{hp}"))
            V = [qkvp.tile([128, ELOC], BF16, name=f"v{st}") for st in range(NKT)]

            def make_v(psq, st):
                ps = psq.tile([128, ELOC], F32)
                c, o = st // 4, 128 * (st % 4)
                for kt in range(NDT):
                    nc.tensor.matmul(
                        ps[:],
                        lhsT=xc[kt][c][:, o:o + 128],
                        rhs=wv_sb[kt][:],
                        start=(kt == 0), stop=(kt == NDT - 1))
                nc.vector.tensor_add(V[st][:], ps[:], bvb_sb[:])

            def make_qtkt_chunk(ps, hp, qk, c):
                dst, wsb = (QT, wq_sb) if qk == 0 else (KT, wk_sb)
                for kt in range(NDT):
                    nc.tensor.matmul(
                        ps[:],
                        lhsT=wsb[kt][:, 128 * hp:128 * (hp + 1)],
                        rhs=xc[kt][c][:],
                        start=(kt == 0), stop=(kt == NDT - 1))
                nc.vector.tensor_scalar_add(
                    dst[hp][:, QC * c:QC * (c + 1)], ps[:],
                    bqk_sb[:, qk, hp:hp + 1])

            def make_qtkt(psq, hp):
                for qk in range(2):
                    for c in range(NQC):
                        ps = psq.tile([128, QC], F32)
                        make_qtkt_chunk(ps, hp, qk, c)

            with tc.tile_pool(name="psq", bufs=6, space="PSUM") as psq:
                # PE warmup: dummy matmuls with no DMA deps fill the initial
                # DMA wait and un-throttle the HAM clock gate early.
                for _ in range(64):
                    t = psq.tile([128, ELOC], F32, name="ps")
                    nc.tensor.matmul(t[:, 0:128], lhsT=ident[:],
                                     rhs=maskadd[:], start=True, stop=True)
                for st in range(NKT):
                    make_v(psq, st)
                make_qtkt(psq, 0)
                make_qtkt(psq, 1)

            # ---- attention + output projection ----
            # The whole attention runs as one flattened, software-pipelined
            # stream of (j, pair, i) steps with a 1-step skew: the score
            # matmuls + exp of step n+1 are issued before the AV matmuls of
            # step n, so the in-order PE queue never bubbles on the exp
            # feedback (bubbles re-throttle the HAM clock gate to 1.2 GHz).
            # The skew also spans stage boundaries, covering the softmax
            # normalization chain with the next stage's score matmuls.
            # Denominators: DVE accumulates the exp tiles per (pair, hp) in
            # bf16; at the stage boundary four 32-row ones-matmuls collapse
            # the partition axis into psd rows 0/32/64/96 (plus tiny positive
            # filler rows) for the reciprocal + broadcast.
            with tc.tile_pool(name="pss", bufs=2, space="PSUM") as pss, \
                 tc.tile_pool(name="psov", bufs=2, space="PSUM") as psov, \
                 tc.tile_pool(name="psd", bufs=1, space="PSUM") as psd, \
                 tc.tile_pool(name="psop", bufs=1, space="PSUM") as psop:

                state = {}   # (j, pair) -> dict(ps_ot, acc, otn)

                def emit_score(j, pair, i):
                    hps = (2 * pair, 2 * pair + 1)
                    diag = i >= 4 * j
                    w = 128 * (i - 4 * j) if diag else 0
                    pts = {}
                    for hp in hps:
                        ps_s = pss.tile([128, 2, QC], F32, tag="pss",
                                        name="ps_s")
                        pt = ptp.tile([128, 2, QC], BF16, tag="pt", name="pt")
                        for h2 in range(2):
                            nc.tensor.matmul(
                                ps_s[:, h2, w:QC],
                                lhsT=KT[hp][64 * h2:64 * (h2 + 1),
                                            128 * i:128 * (i + 1)],
                                rhs=QT[hp][64 * h2:64 * (h2 + 1),
                                           QC * j + w:QC * (j + 1)],
                                start=True, stop=not diag)
                            if diag:
                                # fold the causal mask into the score psum:
                                # += I.T @ maskadd = -160 above the diagonal
                                nc.tensor.matmul(
                                    ps_s[:, h2, w:w + 128],
                                    lhsT=ident[:], rhs=maskadd[:],
                                    start=False, stop=True,
                                    skip_group_check=True)
                        nc.scalar.activation(pt[:, :, w:QC], ps_s[:, :, w:QC],
                                             EXP, scale=SCALE)
                        pts[hp] = pt
                    return pts

                def emit_av(j, pair, i, pts):
                    hps = (2 * pair, 2 * pair + 1)
                    diag = i >= 4 * j
                    w = 128 * (i - 4 * j) if diag else 0
                    nkt = 4 * j + 4
                    last = (i == nkt - 1)
                    if i == 0:
                        state[(j, pair)] = {
                            "ps_ot": {hp: psov.tile([128, QC], F32,
                                                    tag="psov",
                                                    name=f"ps_ot{hp}")
                                      for hp in hps},
                            "acc": {hp: accp.tile([128, 2, QC], BF16,
                                                  tag="acc", name=f"acc{hp}")
                                    for hp in hps},
                        }
                    st = state[(j, pair)]
                    if last:
                        # the last k-tile's exp goes straight into the
                        # denominator psum at the boundary (PE accumulate),
                        # keeping the final DVE add off the critical path
                        st["pt_last"] = pts
                    for hp in hps:
                        pt = pts[hp]
                        if i == 0:
                            nc.vector.tensor_copy(st["acc"][hp][:], pt[:])
                        elif not last:
                            nc.vector.tensor_add(
                                st["acc"][hp][:, :, w:QC],
                                st["acc"][hp][:, :, w:QC], pt[:, :, w:QC])
                        for h2 in range(2):
                            nc.tensor.matmul(
                                st["ps_ot"][hp][64 * h2:64 * (h2 + 1), w:QC],
                                lhsT=V[i][:, 64 * (2 * hp + h2):
                                          64 * (2 * hp + h2 + 1)],
                                rhs=pt[:, h2, w:QC],
                                start=(i == 0), stop=last,
                                tile_position=(0, 64 * h2))

                def emit_boundary(j, pair):
                    # collapse acc along partitions into psd rows 0/32/64/96,
                    # reciprocal straight off PSUM, then per-hp broadcast
                    # matmuls reusing the psd bank.
                    hps = (2 * pair, 2 * pair + 1)
                    st = state.pop((j, pair))
                    ps_d = psd.tile([128, QC], F32, tag="psd", name="ps_d")
                    for ih, hp in enumerate(hps):
                        for h2 in range(2):
                            r = 64 * ih + 32 * h2
                            nc.tensor.matmul(
                                ps_d[r:r + 32, :], lhsT=ones_a[:],
                                rhs=st["acc"][hp][:, h2, :],
                                start=True, stop=False,
                                tile_position=(0, r))
                            nc.tensor.matmul(
                                ps_d[r:r + 32, 384:QC], lhsT=ones_a[:],
                                rhs=st["pt_last"][hp][:, h2, 384:QC],
                                start=False, stop=True,
                                tile_position=(0, r),
                                skip_group_check=True)
                    drf = drp.tile([98, QC], F32, name="drf")
                    nc.vector.reciprocal_approx_fast(drf[:], ps_d[0:98, :])
                    dr = drp.tile([98, QC], BF16, name="dr")
                    with nc.allow_low_precision(reason="denom bf16"):
                        nc.vector.tensor_copy(dr[:], drf[:])
                    for ih, hp in enumerate(hps):
                        bc = bc_x if ih == 0 else bc_y
                        nc.tensor.matmul(ps_d[:], lhsT=bc[:], rhs=dr[:],
                                         start=True, stop=True)
                        dbc = drp.tile([128, QC], BF16, name="dbc")
                        nc.vector.tensor_copy(dbc[:], ps_d[:])
                        otn = otnp.tile([128, QC], BF16, tag="otn", name="otn")
                        nc.vector.tensor_mul(otn[:], st["ps_ot"][hp][:],
                                             dbc[:])
                        otn_j[hp] = otn

                op_queue = []   # pending out-projection group closures

                def out_proj_group(j, m, eo, otns):
                    s0 = QC * j + 128 * m
                    if (2 * m + eo) % 2 == 0:
                        ps_o = psop.tile([128, 512], F32, tag="ps",
                                         name="ps_o")
                    else:
                        ps_o = psd.tile([128, 512], F32, tag="psd",
                                        name="ps_o")
                    for hp in range(NHP):
                        nc.tensor.matmul(
                            ps_o[:],
                            lhsT=otns[hp][:, 128 * m:128 * (m + 1)],
                            rhs=wo_sb[hp][:, 512 * eo:512 * (eo + 1)],
                            start=(hp == 0), stop=(hp == NHP - 1))
                    osb = osbp.tile([128, 512], F32)
                    if j == 0:
                        # tail: ACT is idle after the last exp
                        nc.scalar.copy(osb[:], ps_o[:])
                    else:
                        nc.vector.tensor_copy(osb[:], ps_o[:])
                    nc.sync.dma_start(
                        outp[s0:s0 + 128, 512 * eo:512 * (eo + 1)], osb[:])

                def out_proj(j):
                    # queue this q-chunk's projection groups; the main loop
                    # disperses one per attention step of the next stage so
                    # the PE gets independent filler between score and AV
                    # matmuls instead of a block with psum-drain stalls.
                    otns = dict(otn_j)
                    for m in range(4):
                        for eo in range(2):
                            op_queue.append((j, m, eo, otns))

                seq = [(j, pair, i)
                       for j in (3, 2, 1, 0)
                       for pair in (0, 1)
                       for i in range(4 * j + 4)]
                otn_j = {}

                def retire(step):
                    # emit the AV matmuls of a completed score step, plus the
                    # stage-boundary work when it was the stage's last step
                    (pj, pp, pi), ppts = step
                    emit_av(pj, pp, pi, ppts)
                    if pi == 4 * pj + 3:
                        emit_boundary(pj, pp)
                        if pp == 1:
                            out_proj(pj)

                # Deferred QT/KT (hp 2,3) chunks, one per attention step of
                # the j=3 ramp: each chunk's psum drain is covered by a full
                # attention step and every step gets independent PE filler.
                # Ordered so pair 1's score matmuls find their chunks ready:
                # Q chunk 3 + K chunks 0-2 during (3,0), the rest in (3,1).
                qtkt_sched = {}
                _chunks = [(2, 0, 3), (3, 0, 3), (2, 1, 0), (3, 1, 0),
                           (2, 1, 1), (3, 1, 1), (2, 1, 2), (3, 1, 2),
                           (2, 1, 3), (3, 1, 3), (2, 0, 0), (3, 0, 0),
                           (2, 0, 1), (3, 0, 1), (2, 0, 2), (3, 0, 2)]
                _slots = [(3, 0, i) for i in range(1, 16)] + [(3, 1, 0)]
                for _slot, _ch in zip(_slots, _chunks):
                    qtkt_sched[_slot] = _ch

                from collections import deque
                pending = deque()
                SKEW = 1
                nchunk = [0]
                for stp in seq:
                    if stp in qtkt_sched:
                        hp_, qk_, c_ = qtkt_sched[stp]
                        if nchunk[0] % 2 == 0:
                            ps = psop.tile([128, QC], F32, tag="ps",
                                           name="ps_o")
                        else:
                            ps = psd.tile([128, QC], F32, tag="psd",
                                          name="ps_d")
                        nchunk[0] += 1
                        make_qtkt_chunk(ps, hp_, qk_, c_)
                    pts = emit_score(*stp)
                    if op_queue:
                        out_proj_group(*op_queue.pop(0))
                    pending.append((stp, pts))
                    if len(pending) > SKEW:
                        retire(pending.popleft())
                while pending:
                    retire(pending.popleft())
                while op_queue:
                    out_proj_group(*op_queue.pop(0))
    nc.compile()
    return nc


def _get_nc():
    if "nc" not in _CACHE:
        _CACHE["nc"] = _build_nc()
    return _CACHE["nc"]


def _prep_core_inputs(x, w_qkv, b_qkv, w_out, b, hg):
    r0 = ELOC * hg
    wq = w_qkv[r0:r0 + ELOC, :]
    wk = w_qkv[D + r0:D + r0 + ELOC, :]
    wv = w_qkv[2 * D + r0:2 * D + r0 + ELOC, :]
    bq = b_qkv[r0:r0 + ELOC]
    bk = b_qkv[D + r0:D + r0 + ELOC]
    bv = b_qkv[2 * D + r0:2 * D + r0 + ELOC]

    bf = ml_dtypes.bfloat16
    bqk_arr = np.empty((128, 2, NHP), np.float32)
    bqk_arr[:, 0, :] = bq.reshape(NHP, 128).T
    bqk_arr[:, 1, :] = bk.reshape(NHP, 128).T
    return {
        "xT": np.ascontiguousarray(x[b].T).astype(bf),
        "wqT": np.ascontiguousarray(wq.T).astype(bf),
        "wkT": np.ascontiguousarray(wk.T).astype(bf),
        "wvT": np.ascontiguousarray(wv.T).astype(bf),
        "woT": np.ascontiguousarray(w_out[:, r0:r0 + ELOC].T).astype(bf),
        "bqk": bqk_arr,
        "bvb": np.tile(bv.astype(np.float32)[None, :], (128, 1)),
    }


def kernel(x, w_qkv, b_qkv, w_out, b_out, _trace=False, _trace_kwargs=None):
    x = np.asarray(x, np.float32)
    w_qkv = np.asarray(w_qkv, np.float32)
    b_qkv = np.asarray(b_qkv, np.float32)
    w_out = np.asarray(w_out, np.float32)
    b_out = np.asarray(b_out, np.float32)

    nc = _get_nc()
    in_maps = []
    for core in range(NCORES):
        b, hg = core // 2, core % 2
        in_maps.append(_prep_core_inputs(x, w_qkv, b_qkv, w_out, b, hg))

    kw = {}
    if _trace:
        kw.update(trace=True, **(_trace_kwargs or {}))
    import time
    res = None
    for attempt in range(4):
        try:
            res = bass_utils.run_bass_kernel_spmd(
                nc, in_maps, core_ids=list(range(NCORES)), **kw)
            break
        except Exception:
            if attempt == 3:
                raise
            # Transient axon/NRT device flake: reset the PJRT backend so the
            # retry starts from a clean client, like a fresh process would.
            try:
                import jax
                jax.clear_caches()
                import jax._src.xla_bridge as _xb
                _xb._clear_backends()
            except Exception:
                pass
            time.sleep(5.0 * (attempt + 1))

    out = np.empty((B, S, D), np.float32)
    for b in range(B):
        out[b] = res.results[2 * b]["outp"] + res.results[2 * b + 1]["outp"] \
            + b_out[None, :]
    if _trace:
        return out, res
    return out


# revision 31
# speedup vs baseline: 1.2147x; 1.0193x over previous
"""Causal multi-head attention (dense transformer block) on 8 Trainium2 cores.

Problem: x[4, 2048, 1024], 16 heads, head_dim 64, causal softmax attention
with QKV + output projections (torch Linear layout weights).

Sharding: 8 cores = 4 batches x 2 head-groups (8 heads each).  Each core
computes QKV projection for its 8 heads, attention, and its partial output
projection (row-parallel over w_out).  Host sums the two partials per batch
and adds b_out.

All device layouts are "transposed" so no on-device transposes are needed:
  - x is fed as xT [d, s]; Q^T/K^T are produced as [head_dim, s]
  - scores are computed as S^T [k, q]; the causal mask is folded into the
    score PSUM as an additive -160 block (identity-matmul accumulate), so
    exp(scale*(s-160)) ~ 0 on masked slots and no separate mask multiply
    is needed.
  - softmax denominators are accumulated on the vector engine (bf16 adds of
    the exp tiles) and collapsed with four 1-row ones-matmuls per head-pair
    boundary; this keeps the PE free of the per-k-tile ones-matmul streams.
  - O is accumulated directly as O^T [e_loc, q], which is exactly the lhsT
    the output projection needs.
Matmul inputs are bf16 (PSUM accumulation is fp32); everything else fp32.
"""

import sys

sys.path.insert(0, "/opt/trn_rl_repo")

import numpy as np
import ml_dtypes

import concourse.bass as bass
import concourse.mybir as mybir
import concourse.tile as tile
from concourse import bacc
from concourse import bass_utils
from concourse.masks import make_identity

F32 = mybir.dt.float32
BF16 = mybir.dt.bfloat16
EXP = mybir.ActivationFunctionType.Exp

B, S, D = 4, 2048, 1024
HTOT, HD = 16, 64
NCORES = 8
HLOC = HTOT // 2          # heads per core
ELOC = HLOC * HD          # 512 local embedding width
NHP = HLOC // 2           # 4 head pairs
QC = 512                  # q-chunk width
NQC = S // QC             # 4
NKT = S // 128            # 16 k tiles over sequence
NDT = D // 128            # 8 k tiles over model dim
SCALE = 1.0 / float(np.sqrt(HD))
MASKV = -160.0            # additive causal mask (exp(scale*-160) ~ 2e-9)

_CACHE = {}


def _build_nc():
    nc = bacc.Bacc("TRN2", target_bir_lowering=False, debug=False)

    xT = nc.dram_tensor("xT", [D, S], BF16, kind="ExternalInput")
    wqT = nc.dram_tensor("wqT", [D, ELOC], BF16, kind="ExternalInput")
    wkT = nc.dram_tensor("wkT", [D, ELOC], BF16, kind="ExternalInput")
    wvT = nc.dram_tensor("wvT", [D, ELOC], BF16, kind="ExternalInput")
    woT = nc.dram_tensor("woT", [ELOC, D], BF16, kind="ExternalInput")
    bqk = nc.dram_tensor("bqk", [128, 2, NHP], F32, kind="ExternalInput")
    bvb = nc.dram_tensor("bvb", [128, ELOC], F32, kind="ExternalInput")
    outp = nc.dram_tensor("outp", [S, D], F32, kind="ExternalOutput")

    with tile.TileContext(nc) as tc:
        with tc.tile_pool(name="const", bufs=1) as constp, \
             tc.tile_pool(name="wpool", bufs=1) as wp, \
             tc.tile_pool(name="qkv", bufs=1) as qkvp, \
             tc.tile_pool(name="xt", bufs=1) as xtp, \
             tc.tile_pool(name="pt", bufs=12) as ptp, \
             tc.tile_pool(name="acc", bufs=4) as accp, \
             tc.tile_pool(name="otn", bufs=8) as otnp, \
             tc.tile_pool(name="dr", bufs=6) as drp, \
             tc.tile_pool(name="osb", bufs=6) as osbp:

            # ---- constants (all gpsimd, no DMA deps) ----
            ident = constp.tile([128, 128], BF16, name="ident")
            make_identity(nc, ident[:])
            # additive causal mask in S^T [k, q] orientation: 0 where k <= q,
            # MASKV where k > q  (keep where -k + q >= 0)
            maskadd = constp.tile([128, 128], BF16, name="maskadd")
            nc.gpsimd.memset(maskadd[:], 0.0)
            nc.gpsimd.affine_select(
                out=maskadd[:], in_=maskadd[:],
                compare_op=mybir.AluOpType.is_ge,
                fill=MASKV, base=0,
                pattern=[[1, 128]], channel_multiplier=-1)
            # 32-wide ones block: col 0 sums the partition axis, cols 1-31
            # (value 1e-8) overwrite the other psd rows with tiny positives
            # so the boundary reciprocal never sees stale garbage (the psd
            # bank is shared with out-projection psums).
            ones_a = constp.tile([128, 32], BF16, name="ones_a")
            nc.gpsimd.memset(ones_a[:], 1e-8)
            nc.gpsimd.memset(ones_a[:, 0:1], 1.0)
            bc_x = constp.tile([98, 128], BF16, name="bc_x")
            nc.gpsimd.memset(bc_x[:], 0.0)
            nc.gpsimd.memset(bc_x[0:1, 0:64], 1.0)
            nc.gpsimd.memset(bc_x[32:33, 64:128], 1.0)
            bc_y = constp.tile([98, 128], BF16, name="bc_y")
            nc.gpsimd.memset(bc_y[:], 0.0)
            nc.gpsimd.memset(bc_y[64:65, 0:64], 1.0)
            nc.gpsimd.memset(bc_y[96:97, 64:128], 1.0)

            # ---- weights + xT.  Order matters: V weights and the first x
            # column-chunk land first so compute starts ~6us in; the rest of
            # x streams in column-chunks consumed st-by-st by make_v.  DMA
            # issue itself costs ~0.6us/descriptor per engine queue, so the
            # input loads round-robin across three issuing engines. ----
            _dma_engs = (nc.sync, nc.scalar, nc.gpsimd)
            _dma_n = [0]

            def dma_in(dst, src):
                _dma_engs[_dma_n[0] % 3].dma_start(dst, src)
                _dma_n[0] += 1

            bqk_sb = constp.tile([128, 2, NHP], F32, name="bqk_sb")
            dma_in(bqk_sb[:], bqk[:])
            bvb_sb = constp.tile([128, ELOC], F32, name="bvb_sb")
            dma_in(bvb_sb[:], bvb[:])

            wv_sb = []
            for kt in range(NDT):
                t = wp.tile([128, ELOC], BF16, name=f"wv{kt}")
                dma_in(t[:], wvT[128 * kt:128 * (kt + 1), :])
                wv_sb.append(t)
            xc = [[None] * NQC for _ in range(NDT)]
            for c in range(NQC):
                for kt in range(NDT):
                    t = xtp.tile([128, QC], BF16, name=f"x{kt}_{c}")
                    dma_in(
                        t[:], xT[128 * kt:128 * (kt + 1), QC * c:QC * (c + 1)])
                    xc[kt][c] = t
            wq_sb, wk_sb = [], []
            for kt in range(NDT):
                for lst, srct, nm in ((wq_sb, wqT, "wq"), (wk_sb, wkT, "wk")):
                    t = wp.tile([128, ELOC], BF16, name=f"{nm}{kt}")
                    dma_in(t[:], srct[128 * kt:128 * (kt + 1), :])
                    lst.append(t)
            wo_sb = []
            for hp in range(NHP):
                t = wp.tile([128, D], BF16, name=f"wo{hp}")
                dma_in(t[:], woT[128 * hp:128 * (hp + 1), :])
                wo_sb.append(t)

            # ---- QKV projection ----
            QT, KT = [], []
            for hp in range(NHP):
                QT.append(qkvp.tile([128, S], BF16, name=f"qt{hp}"))
                KT.append(qkvp.tile([128, S], BF16, name=f"kt{hp}"))
            V = [qkvp.tile([128, ELOC], BF16, name=f"v{st}") for st in range(NKT)]

            def make_v(psq, st):
                ps = psq.tile([128, ELOC], F32)
                c, o = st // 4, 128 * (st % 4)
                for kt in range(NDT):
                    nc.tensor.matmul(
                        ps[:],
                        lhsT=xc[kt][c][:, o:o + 128],
                        rhs=wv_sb[kt][:],
                        start=(kt == 0), stop=(kt == NDT - 1))
                nc.vector.tensor_add(V[st][:], ps[:], bvb_sb[:])

            def make_qtkt_chunk(ps, hp, qk, c):
                dst, wsb = (QT, wq_sb) if qk == 0 else (KT, wk_sb)
                for kt in range(NDT):
                    nc.tensor.matmul(
                        ps[:],
                        lhsT=wsb[kt][:, 128 * hp:128 * (hp + 1)],
                        rhs=xc[kt][c][:],
                        start=(kt == 0), stop=(kt == NDT - 1))
                nc.vector.tensor_scalar_add(
                    dst[hp][:, QC * c:QC * (c + 1)], ps[:],
                    bqk_sb[:, qk, hp:hp + 1])

            def make_qtkt(psq, hp):
                for qk in range(2):
                    for c in range(NQC):
                        ps = psq.tile([128, QC], F32)
                        make_qtkt_chunk(ps, hp, qk, c)

            with tc.tile_pool(name="psq", bufs=6, space="PSUM") as psq:
                # PE warmup: dummy matmuls with no DMA deps fill the initial
                # DMA wait and un-throttle the HAM clock gate early.
                for _ in range(64):
                    t = psq.tile([128, ELOC], F32, name="ps")
                    nc.tensor.matmul(t[:, 0:128], lhsT=ident[:],
                                     rhs=maskadd[:], start=True, stop=True)
                for st in range(NKT):
                    make_v(psq, st)
                make_qtkt(psq, 0)
                make_qtkt(psq, 1)

            # ---- attention + output projection ----
            # The whole attention runs as one flattened, software-pipelined
            # stream of (j, pair, i) steps with a 1-step skew: the score
            # matmuls + exp of step n+1 are issued before the AV matmuls of
            # step n, so the in-order PE queue never bubbles on the exp
            # feedback (bubbles re-throttle the HAM clock gate to 1.2 GHz).
            # The skew also spans stage boundaries, covering the softmax
            # normalization chain with the next stage's score matmuls.
            # Denominators: DVE accumulates the exp tiles per (pair, hp) in
            # bf16; at the stage boundary four 32-row ones-matmuls collapse
            # the partition axis into psd rows 0/32/64/96 (plus tiny positive
            # filler rows) for the reciprocal + broadcast.
            with tc.tile_pool(name="pss", bufs=2, space="PSUM") as pss, \
                 tc.tile_pool(name="psov", bufs=2, space="PSUM") as psov, \
                 tc.tile_pool(name="psd", bufs=1, space="PSUM") as psd, \
                 tc.tile_pool(name="psop", bufs=1, space="PSUM") as psop:

                state = {}   # (j, pair) -> dict(ps_ot, acc, otn)

                def emit_score(j, pair, i):
                    hps = (2 * pair, 2 * pair + 1)
                    diag = i >= 4 * j
                    w = 128 * (i - 4 * j) if diag else 0
                    pts = {}
                    for hp in hps:
                        ps_s = pss.tile([128, 2, QC], F32, tag="pss",
                                        name="ps_s")
                        pt = ptp.tile([128, 2, QC], BF16, tag="pt", name="pt")
                        for h2 in range(2):
                            nc.tensor.matmul(
                                ps_s[:, h2, w:QC],
                                lhsT=KT[hp][64 * h2:64 * (h2 + 1),
                                            128 * i:128 * (i + 1)],
                                rhs=QT[hp][64 * h2:64 * (h2 + 1),
                                           QC * j + w:QC * (j + 1)],
                                start=True, stop=not diag)
                            if diag:
                                # fold the causal mask into the score psum:
                                # += I.T @ maskadd = -160 above the diagonal
                                nc.tensor.matmul(
                                    ps_s[:, h2, w:w + 128],
                                    lhsT=ident[:], rhs=maskadd[:],
                                    start=False, stop=True,
                                    skip_group_check=True)
                        nc.scalar.activation(pt[:, :, w:QC], ps_s[:, :, w:QC],
                                             EXP, scale=SCALE)
                        pts[hp] = pt
                    return pts

                def emit_av(j, pair, i, pts):
                    hps = (2 * pair, 2 * pair + 1)
                    diag = i >= 4 * j
                    w = 128 * (i - 4 * j) if diag else 0
                    nkt = 4 * j + 4
                    last = (i == nkt - 1)
                    if i == 0:
                        state[(j, pair)] = {
                            "ps_ot": {hp: psov.tile([128, QC], F32,
                                                    tag="psov",
                                                    name=f"ps_ot{hp}")
                                      for hp in hps},
                            "acc": {hp: accp.tile([128, 2, QC], BF16,
                                                  tag="acc", name=f"acc{hp}")
                                    for hp in hps},
                        }
                    st = state[(j, pair)]
                    if last:
                        # the last k-tile's exp goes straight into the
                        # denominator psum at the boundary (PE accumulate),
                        # keeping the final DVE add off the critical path
                        st["pt_last"] = pts
                    for hp in hps:
                        pt = pts[hp]
                        if i == 0:
                            nc.vector.tensor_copy(st["acc"][hp][:], pt[:])
                        elif not last:
                            nc.vector.tensor_add(
                                st["acc"][hp][:, :, w:QC],
                                st["acc"][hp][:, :, w:QC], pt[:, :, w:QC])
                        for h2 in range(2):
                            nc.tensor.matmul(
                                st["ps_ot"][hp][64 * h2:64 * (h2 + 1), w:QC],
                                lhsT=V[i][:, 64 * (2 * hp + h2):
                                          64 * (2 * hp + h2 + 1)],
                                rhs=pt[:, h2, w:QC],
                                start=(i == 0), stop=last,
                                tile_position=(0, 64 * h2))

                def emit_boundary(j, pair):
                    # collapse acc along partitions into psd rows 0/32/64/96,
                    # reciprocal straight off PSUM, then per-hp broadcast
                    # matmuls reusing the psd bank.
                    hps = (2 * pair, 2 * pair + 1)
                    st = state.pop((j, pair))
                    ps_d = psd.tile([128, QC], F32, tag="psd", name="ps_d")
                    for ih, hp in enumerate(hps):
                        for h2 in range(2):
                            r = 64 * ih + 32 * h2
                            nc.tensor.matmul(
                                ps_d[r:r + 32, :], lhsT=ones_a[:],
                                rhs=st["acc"][hp][:, h2, :],
                                start=True, stop=False,
                                tile_position=(0, r))
                            nc.tensor.matmul(
                                ps_d[r:r + 32, 384:QC], lhsT=ones_a[:],
                                rhs=st["pt_last"][hp][:, h2, 384:QC],
                                start=False, stop=True,
                                tile_position=(0, r),
                                skip_group_check=True)
                    drf = drp.tile([98, QC], F32, name="drf")
                    nc.vector.reciprocal_approx_fast(drf[:], ps_d[0:98, :])
                    dr = drp.tile([98, QC], BF16, name="dr")
                    with nc.allow_low_precision(reason="denom bf16"):
                        nc.vector.tensor_copy(dr[:], drf[:])
                    for ih, hp in enumerate(hps):
                        bc = bc_x if ih == 0 else bc_y
                        nc.tensor.matmul(ps_d[:], lhsT=bc[:], rhs=dr[:],
                                         start=True, stop=True)
                        dbc = drp.tile([128, QC], BF16, name="dbc")
                        nc.vector.tensor_copy(dbc[:], ps_d[:])
                        otn = otnp.tile([128, QC], BF16, tag="otn", name="otn")
                        nc.vector.tensor_mul(otn[:], st["ps_ot"][hp][:],
                                             dbc[:])
                        otn_j[hp] = otn

                op_queue = []   # pending out-projection group closures

                def out_proj_group(j, m, eo, otns):
                    s0 = QC * j + 128 * m
                    if (2 * m + eo) % 2 == 0:
                        ps_o = psop.tile([128, 512], F32, tag="ps",
                                         name="ps_o")
                    else:
                        ps_o = psd.tile([128, 512], F32, tag="psd",
                                        name="ps_o")
                    for hp in range(NHP):
                        nc.tensor.matmul(
                            ps_o[:],
                            lhsT=otns[hp][:, 128 * m:128 * (m + 1)],
                            rhs=wo_sb[hp][:, 512 * eo:512 * (eo + 1)],
                            start=(hp == 0), stop=(hp == NHP - 1))
                    osb = osbp.tile([128, 512], F32)
                    if j == 0:
                        # tail: ACT is idle after the last exp
                        nc.scalar.copy(osb[:], ps_o[:])
                    else:
                        nc.vector.tensor_copy(osb[:], ps_o[:])
                    nc.sync.dma_start(
                        outp[s0:s0 + 128, 512 * eo:512 * (eo + 1)], osb[:])

                def out_proj(j):
                    # queue this q-chunk's projection groups; the main loop
                    # disperses one per attention step of the next stage so
                    # the PE gets independent filler between score and AV
                    # matmuls instead of a block with psum-drain stalls.
                    otns = dict(otn_j)
                    for m in range(4):
                        for eo in range(2):
                            op_queue.append((j, m, eo, otns))

                seq = [(j, pair, i)
                       for j in (3, 2, 1, 0)
                       for pair in (0, 1)
                       for i in range(4 * j + 4)]
                otn_j = {}

                def retire(step):
                    # emit the AV matmuls of a completed score step, plus the
                    # stage-boundary work when it was the stage's last step
                    (pj, pp, pi), ppts = step
                    emit_av(pj, pp, pi, ppts)
                    if pi == 4 * pj + 3:
                        emit_boundary(pj, pp)
                        if pp == 1:
                            out_proj(pj)

                # Deferred QT/KT (hp 2,3) chunks, one per attention step of
                # the j=3 ramp: each chunk's psum drain is covered by a full
                # attention step and every step gets independent PE filler.
                # Ordered so pair 1's score matmuls find their chunks ready:
                # Q chunk 3 + K chunks 0-2 during (3,0), the rest in (3,1).
                qtkt_sched = {}
                _chunks = [(2, 0, 3), (3, 0, 3), (2, 1, 0), (3, 1, 0),
                           (2, 1, 1), (3, 1, 1), (2, 1, 2), (3, 1, 2),
                           (2, 1, 3), (3, 1, 3), (2, 0, 0), (3, 0, 0),
                           (2, 0, 1), (3, 0, 1), (2, 0, 2), (3, 0, 2)]
                _slots = [(3, 0, i) for i in range(8, 16)] + \
                         [(3, 1, i) for i in range(8)]
                for _slot, _ch in zip(_slots, _chunks):
                    qtkt_sched[_slot] = _ch

                from collections import deque
                pending = deque()
                SKEW = 1
                nchunk = [0]
                for stp in seq:
                    if stp in qtkt_sched:
                        hp_, qk_, c_ = qtkt_sched[stp]
                        if nchunk[0] % 2 == 0:
                            ps = psop.tile([128, QC], F32, tag="ps",
                                           name="ps_o")
                        else:
                            ps = psd.tile([128, QC], F32, tag="psd",
                                          name="ps_d")
                        nchunk[0] += 1
                        make_qtkt_chunk(ps, hp_, qk_, c_)
                    pts = emit_score(*stp)
                    if op_queue:
                        out_proj_group(*op_queue.pop(0))
                    pending.append((stp, pts))
                    if len(pending) > SKEW:
                        retire(pending.popleft())
                while pending:
                    retire(pending.popleft())
                while op_queue:
                    out_proj_group(*op_queue.pop(0))
    nc.compile()
    return nc


def _get_nc():
    if "nc" not in _CACHE:
        _CACHE["nc"] = _build_nc()
    return _CACHE["nc"]


def _prep_core_inputs(x, w_qkv, b_qkv, w_out, b, hg):
    r0 = ELOC * hg
    wq = w_qkv[r0:r0 + ELOC, :]
    wk = w_qkv[D + r0:D + r0 + ELOC, :]
    wv = w_qkv[2 * D + r0:2 * D + r0 + ELOC, :]
    bq = b_qkv[r0:r0 + ELOC]
    bk = b_qkv[D + r0:D + r0 + ELOC]
    bv = b_qkv[2 * D + r0:2 * D + r0 + ELOC]

    bf = ml_dtypes.bfloat16
    bqk_arr = np.empty((128, 2, NHP), np.float32)
    bqk_arr[:, 0, :] = bq.reshape(NHP, 128).T
    bqk_arr[:, 1, :] = bk.reshape(NHP, 128).T
    return {
        "xT": np.ascontiguousarray(x[b].T).astype(bf),
        "wqT": np.ascontiguousarray(wq.T).astype(bf),
        "wkT": np.ascontiguousarray(wk.T).astype(bf),
        "wvT": np.ascontiguousarray(wv.T).astype(bf),
        "woT": np.ascontiguousarray(w_out[:, r0:r0 + ELOC].T).astype(bf),
        "bqk": bqk_arr,
        "bvb": np.tile(bv.astype(np.float32)[None, :], (128, 1)),
    }


def kernel(x, w_qkv, b_qkv, w_out, b_out, _trace=False, _trace_kwargs=None):
    x = np.asarray(x, np.float32)
    w_qkv = np.asarray(w_qkv, np.float32)
    b_qkv = np.asarray(b_qkv, np.float32)
    w_out = np.asarray(w_out, np.float32)
    b_out = np.asarray(b_out, np.float32)

    nc = _get_nc()
    in_maps = []
    for core in range(NCORES):
        b, hg = core // 2, core % 2
        in_maps.append(_prep_core_inputs(x, w_qkv, b_qkv, w_out, b, hg))

    kw = {}
    if _trace:
        kw.update(trace=True, **(_trace_kwargs or {}))
    import time
    res = None
    for attempt in range(4):
        try:
            res = bass_utils.run_bass_kernel_spmd(
                nc, in_maps, core_ids=list(range(NCORES)), **kw)
            break
        except Exception:
            if attempt == 3:
                raise
            # Transient axon/NRT device flake: reset the PJRT backend so the
            # retry starts from a clean client, like a fresh process would.
            try:
                import jax
                jax.clear_caches()
                import jax._src.xla_bridge as _xb
                _xb._clear_backends()
            except Exception:
                pass
            time.sleep(5.0 * (attempt + 1))

    out = np.empty((B, S, D), np.float32)
    for b in range(B):
        out[b] = res.results[2 * b]["outp"] + res.results[2 * b + 1]["outp"] \
            + b_out[None, :]
    if _trace:
        return out, res
    return out
